# revision 1
# baseline (speedup 1.0000x reference)
"""DRRN scoring network, v4: per-encoder sharding on 8 NeuronCores.

Two-dispatch design. Phase 1 (encoders): each core runs ONE GRU encoder
over 128 rows x 256 steps -- cores 0-1 obs, 2-3 look, 4-5 inv, 6-7 act
(act runs 8 sequential 32-step chunks of 128 rows). Full-width matmuls
(M=128) replace the baseline's 32-col tiling: 8 matmuls + 2 dma-xbar
transposes per step instead of 27 matmuls + 2 PE transposes. Each core
also computes its encoder's MLP contribution c_e = h @ W_e^T per chunk
slot. Phase 2 (MLP): data-parallel over states; host only slices /
replicates phase-1 outputs between dispatches (no host flops).
"""
import numpy as np
import ml_dtypes
import concourse.bacc as bacc
import concourse.mybir as mybir
from concourse.tile import TileContext
from concourse.bass_utils import run_bass_kernel_spmd

dt = mybir.dt
F32, BF16, I16, F32R = dt.float32, dt.bfloat16, dt.int16, dt.float32r
bf = ml_dtypes.bfloat16

V, E, H = 32000, 128, 256
B, S = 256, 256
A, SA = 8, 32
NCORES = 8
NSTEP = S
NIDX = NSTEP * 128
NSLAB = 8
SLAB = NIDX // NSLAB
H3, H2 = 3 * H, 2 * H

Sig = mybir.ActivationFunctionType.Sigmoid
Tanh = mybir.ActivationFunctionType.Tanh
Relu = mybir.ActivationFunctionType.Relu
Ident = mybir.ActivationFunctionType.Identity
MUL = mybir.AluOpType.mult
ADD = mybir.AluOpType.add
SUB = mybir.AluOpType.subtract


def build_enc(nreps=1):
    nc = bacc.Bacc("TRN2", target_bir_lowering=False, debug=False)

    d_emb = nc.declare_dram_parameter("embb", [V, E], BF16, isOutput=False)
    d_idx = nc.declare_dram_parameter("idx", [128, NIDX // 16], I16, isOutput=False)
    d_wih = nc.declare_dram_parameter("wihT", [E, H3], BF16, isOutput=False)
    d_whh = nc.declare_dram_parameter("whhT", [128, 2, H3], BF16, isOutput=False)
    d_sel = nc.declare_dram_parameter("sel", [4, 128], F32R, isOutput=False)
    d_brz = nc.declare_dram_parameter("brz", [4, H2], F32R, isOutput=False)
    d_bnn = nc.declare_dram_parameter("bnn", [4, H2], F32R, isOutput=False)
    d_m = nc.declare_dram_parameter("mask", [128, NSTEP], F32, isOutput=False)
    d_rm = nc.declare_dram_parameter("rmask", [128, 1], F32, isOutput=False)
    d_rbt = nc.declare_dram_parameter("rbt", [128, 2, 128], BF16, isOutput=False)
    d_hw = nc.declare_dram_parameter("hWT", [128, 2, H], BF16, isOutput=False)
    d_c = nc.declare_dram_parameter("contrib", [A, 128, H], F32, isOutput=True)

    with TileContext(nc) as tc:
        with tc.tile_pool(name="w", bufs=1) as wp, \
             tc.tile_pool(name="xp", bufs=1) as xp, \
             tc.tile_pool(name="st", bufs=1) as stp, \
             tc.tile_pool(name="rot", bufs=3) as rp, \
             tc.tile_pool(name="ps", bufs=2, space="PSUM") as ps:

            t_idx = wp.tile([128, NIDX // 16], I16, name="t_idx")
            nc.sync.dma_start(out=t_idx[:], in_=d_idx[:])
            t_wih = wp.tile([E, H3], BF16, name="t_wih")
            nc.sync.dma_start(out=t_wih[:], in_=d_wih[:])
            t_whh = wp.tile([128, 2, H3], BF16, name="t_whh")
            nc.sync.dma_start(out=t_whh[:], in_=d_whh[:])
            t_sel = wp.tile([4, 128], F32R, name="t_sel")
            nc.sync.dma_start(out=t_sel[:], in_=d_sel[:])
            t_brz = wp.tile([4, H2], F32R, name="t_brz")
            nc.sync.dma_start(out=t_brz[:], in_=d_brz[:])
            t_bnn = wp.tile([4, H2], F32R, name="t_bnn")
            nc.sync.dma_start(out=t_bnn[:], in_=d_bnn[:])
            t_m = wp.tile([128, NSTEP], F32, name="t_m")
            nc.sync.dma_start(out=t_m[:], in_=d_m[:])
            t_rm = wp.tile([128, 1], F32, name="t_rm")
            nc.sync.dma_start(out=t_rm[:], in_=d_rm[:])
            t_rbt = wp.tile([128, 2, 128], BF16, name="t_rbt")
            nc.sync.dma_start(out=t_rbt[:], in_=d_rbt[:])
            t_hw = wp.tile([128, 2, H], BF16, name="t_hw")
            nc.sync.dma_start(out=t_hw[:], in_=d_hw[:])

            slots = [stp.tile([128, 2, 128], BF16, tag=f"slot{g}", name=f"slot{g}")
                     for g in range(A)]

            for rep in range(nreps):
                xts = []
                for s in range(NSLAB):
                    xt = xp.tile([128, 1, SLAB], BF16, tag=f"xt{s}", name=f"xt{s}")
                    nc.gpsimd.dma_gather(
                        out_ap=xt[:], in_ap=d_emb[:],
                        idxs_ap=t_idx[:, (SLAB // 16) * s:(SLAB // 16) * (s + 1)],
                        num_idxs=SLAB, num_idxs_reg=SLAB, elem_size=E,
                        transpose=True, single_packet=False,
                    )
                    xts.append(xt)
                h_A = rp.tile([128, H], BF16, tag="hA", name="hA")
                nc.vector.memset(h_A[:], 0.0)
                h_T = rp.tile([128, 2, 128], BF16, tag="hT", name="hT")
                nc.vector.memset(h_T[:], 0.0)

                for t in range(NSTEP):
                    xT = xts[t // SA][:, 0, (t % SA) * 128:(t % SA + 1) * 128]
                    p1 = ps.tile([128, H2], F32, tag="p1", name="p1")
                    p23 = ps.tile([128, H2], F32, tag="p23", name="p23")
                    nc.tensor.matmul(p1[:], t_sel[:], t_brz[:], start=True, stop=False)
                    nc.tensor.matmul(p23[:], t_sel[:], t_bnn[:], start=True, stop=False)
                    nc.tensor.matmul(p1[:], xT, t_wih[:, 0:H2], start=False, stop=False)
                    nc.tensor.matmul(p23[:, 0:H], xT, t_wih[:, H2:H3],
                                     start=False, stop=True)
                    nc.tensor.matmul(p1[:], h_T[:, 0, :], t_whh[:, 0, 0:H2],
                                     start=False, stop=False)
                    nc.tensor.matmul(p23[:, H:H2], h_T[:, 0, :], t_whh[:, 0, H2:H3],
                                     start=False, stop=False, skip_group_check=True)
                    nc.tensor.matmul(p1[:], h_T[:, 1, :], t_whh[:, 1, 0:H2],
                                     start=False, stop=True)
                    nc.tensor.matmul(p23[:, H:H2], h_T[:, 1, :], t_whh[:, 1, H2:H3],
                                     start=False, stop=True, skip_group_check=True)

                    s_r = rp.tile([128, H], BF16, tag="sr", name="s_r")
                    nc.scalar.activation(s_r[:], p1[:, 0:H], Sig)
                    s_zb = rp.tile([128, H], BF16, tag="szb", name="s_zb")
                    nc.scalar.activation(s_zb[:], p1[:, H:H2], Sig, scale=-1.0)
                    t1 = rp.tile([128, H], BF16, tag="t1", name="t1")
                    nc.vector.tensor_tensor(t1[:], s_r[:], p23[:, H:H2], MUL)
                    t2 = rp.tile([128, H], BF16, tag="t2", name="t2")
                    nc.vector.tensor_tensor(t2[:], t1[:], p23[:, 0:H], ADD)
                    s_n = rp.tile([128, H], BF16, tag="sn", name="s_n")
                    nc.scalar.activation(s_n[:], t2[:], Tanh)
                    s_d = rp.tile([128, H], BF16, tag="sd", name="s_d")
                    nc.vector.tensor_tensor(s_d[:], s_n[:], h_A[:], SUB)
                    s_u = rp.tile([128, H], BF16, tag="su", name="s_u")
                    nc.vector.scalar_tensor_tensor(s_u[:], s_zb[:], t_m[:, t:t + 1],
                                                   s_d[:], MUL, MUL)
                    h_A2 = rp.tile([128, H], BF16, tag="hA", name="hA")
                    nc.vector.tensor_tensor(h_A2[:], h_A[:], s_u[:], ADD)

                    if t % SA == SA - 1:
                        g = t // SA
                        hTp = rp.tile([128, 2, 128], BF16, tag="hTp", name="hTp")
                        nc.sync.dma_start_transpose(out=hTp[:], in_=h_A2[:])
                        nc.vector.tensor_copy(slots[g][:], hTp[:])
                        if t != NSTEP - 1:
                            h_T2 = rp.tile([128, 2, 128], BF16, tag="hT", name="hT")
                            nc.vector.tensor_tensor(h_T2[:], hTp[:], t_rbt[:], MUL)
                            h_T = h_T2
                            h_A3 = rp.tile([128, H], BF16, tag="hA", name="hA")
                            nc.vector.tensor_scalar_mul(h_A3[:], h_A2[:], t_rm[:, 0:1])
                            h_A = h_A3
                    else:
                        h_T2 = rp.tile([128, 2, 128], BF16, tag="hT", name="hT")
                        nc.sync.dma_start_transpose(out=h_T2[:], in_=h_A2[:])
                        h_T = h_T2
                        h_A = h_A2

                for g in range(A):
                    pc = ps.tile([128, H], F32, tag="pc", name="pc")
                    nc.tensor.matmul(pc[:], slots[g][:, 0, :], t_hw[:, 0, :],
                                     start=True, stop=False)
                    nc.tensor.matmul(pc[:], slots[g][:, 1, :], t_hw[:, 1, :],
                                     start=False, stop=True)
                    c_s = rp.tile([128, H], F32, tag="cs", name="c_s")
                    nc.scalar.activation(c_s[:], pc[:], Ident)
                    nc.sync.dma_start(out=d_c[g], in_=c_s[:])

    nc.compile()
    return nc


def build_mlp(nreps=1):
    nc = bacc.Bacc("TRN2", target_bir_lowering=False, debug=False)

    d_obs = nc.declare_dram_parameter("cobs", [128, 2, H], BF16, isOutput=False)
    d_look = nc.declare_dram_parameter("clook", [128, 2, H], BF16, isOutput=False)
    d_inv = nc.declare_dram_parameter("cinv", [128, 2, H], BF16, isOutput=False)
    d_act = nc.declare_dram_parameter("cact", [128, 2, H], BF16, isOutput=False)
    d_hbt = nc.declare_dram_parameter("hbt", [128, H], BF16, isOutput=False)
    d_scb = nc.declare_dram_parameter("scorerb", [128, H], BF16, isOutput=False)
    d_sbt = nc.declare_dram_parameter("sbt", [128, 1], F32, isOutput=False)
    d_q = nc.declare_dram_parameter("q", [2, 128], F32, isOutput=True)

    with TileContext(nc) as tc:
        with tc.tile_pool(name="w", bufs=1) as wp, \
             tc.tile_pool(name="rot", bufs=2) as rp:
            t_obs = wp.tile([128, 2, H], BF16, name="t_obs")
            nc.sync.dma_start(out=t_obs[:], in_=d_obs[:])
            t_look = wp.tile([128, 2, H], BF16, name="t_look")
            nc.sync.dma_start(out=t_look[:], in_=d_look[:])
            t_inv = wp.tile([128, 2, H], BF16, name="t_inv")
            nc.sync.dma_start(out=t_inv[:], in_=d_inv[:])
            t_act = wp.tile([128, 2, H], BF16, name="t_act")
            nc.sync.dma_start(out=t_act[:], in_=d_act[:])
            t_hbt = wp.tile([128, H], BF16, name="t_hbt")
            nc.sync.dma_start(out=t_hbt[:], in_=d_hbt[:])
            t_scb = wp.tile([128, H], BF16, name="t_scb")
            nc.sync.dma_start(out=t_scb[:], in_=d_scb[:])
            t_sbt = wp.tile([128, 1], F32, name="t_sbt")
            nc.sync.dma_start(out=t_sbt[:], in_=d_sbt[:])

            for rep in range(nreps):
                for k in range(2):
                    a1 = rp.tile([128, H], BF16, tag="a1", name="a1")
                    nc.vector.tensor_tensor(a1[:], t_obs[:, k, :], t_look[:, k, :], ADD)
                    a2 = rp.tile([128, H], BF16, tag="a2", name="a2")
                    nc.vector.tensor_tensor(a2[:], t_inv[:, k, :], t_act[:, k, :], ADD)
                    a3 = rp.tile([128, H], BF16, tag="a3", name="a3")
                    nc.vector.tensor_tensor(a3[:], a1[:], a2[:], ADD)
                    zp = rp.tile([128, H], BF16, tag="zp", name="zp")
                    nc.vector.tensor_tensor(zp[:], a3[:], t_hbt[:], ADD)
                    z = rp.tile([128, H], BF16, tag="z", name="z")
                    nc.scalar.activation(z[:], zp[:], Relu)
                    qm = rp.tile([128, H], F32, tag="qm", name="qm")
                    nc.vector.tensor_tensor(qm[:], z[:], t_scb[:], MUL)
                    qv = rp.tile([128, 1], F32, tag="qv", name="qv")
                    nc.vector.reduce_sum(qv[:], qm[:], axis=mybir.AxisListType.X)
                    qf = rp.tile([128, 1], F32, tag="qf", name="qf")
                    nc.vector.tensor_scalar_add(qf[:], qv[:], t_sbt[:, 0:1])
                    nc.sync.dma_start(out=d_q[k], in_=qf[:, 0])

    nc.compile()
    return nc


def _wrap_idx(tokens_flat):
    out = np.zeros((128, NIDX // 16), np.int16)
    for s in range(NSLAB):
        blk = tokens_flat[SLAB * s:SLAB * (s + 1)].reshape(SLAB // 16, 16).T
        out[:, (SLAB // 16) * s:(SLAB // 16) * (s + 1)] = np.tile(blk, (8, 1))
    return out


def prep_enc(obs_tokens, obs_len, look_tokens, look_len, inv_tokens, inv_len,
             act_tokens, act_len, emb, Wih, Whh, bih, bhh,
             hidden_W, hidden_b, scorer_W, scorer_b):
    npf = np.asarray
    enc_tok = [npf(obs_tokens), npf(look_tokens), npf(inv_tokens)]
    enc_len = [np.maximum(npf(obs_len), 1), np.maximum(npf(look_len), 1),
               np.maximum(npf(inv_len), 1)]
    act_tokens = npf(act_tokens)
    act_len = np.maximum(npf(act_len), 1)
    emb = npf(emb, np.float32)
    Wih = npf(Wih, np.float32)
    Whh = npf(Whh, np.float32)
    bih = npf(bih, np.float32)
    bhh = npf(bhh, np.float32)
    hidden_W = npf(hidden_W, np.float32)

    emb_bf = emb.astype(bf)
    sel = np.zeros((4, 128), np.float32)
    sel[0, :] = 1.0

    in_maps = []
    for c in range(NCORES):
        e = c // 2
        hf = c % 2
        wihT = np.ascontiguousarray(Wih[e].T).astype(bf)            # [E, 768]
        whhT = np.ascontiguousarray(
            Whh[e].T.reshape(2, 128, H3).transpose(1, 0, 2)).astype(bf)
        brz = np.zeros((4, H2), np.float32)
        brz[0] = bih[e, 0:H2] + bhh[e, 0:H2]
        bnn = np.zeros((4, H2), np.float32)
        bnn[0, 0:H] = bih[e, H2:H3]
        bnn[0, H:H2] = bhh[e, H2:H3]
        hWT = np.ascontiguousarray(
            hidden_W[:, H * e:H * (e + 1)].T.reshape(2, 128, H)
            .transpose(1, 0, 2)).astype(bf)                         # [128,2,H]

        toks = np.zeros((NSTEP, 128), np.int64)
        m = np.zeros((128, NSTEP), np.float32)
        if e < 3:
            seqs = enc_tok[e][128 * hf:128 * (hf + 1)]               # [128, S]
            lens = enc_len[e][128 * hf:128 * (hf + 1)]
            toks[:, :] = seqs.T
            m[:, :] = (np.arange(NSTEP)[None, :] < lens[:, None])
            rmv = 1.0
        else:
            at = act_tokens[1024 * hf:1024 * (hf + 1)]               # [1024, SA]
            al = act_len[1024 * hf:1024 * (hf + 1)]
            for g in range(A):
                toks[SA * g:SA * (g + 1), :] = at[128 * g:128 * (g + 1)].T
                m[:, SA * g:SA * (g + 1)] = (
                    np.arange(SA)[None, :] < al[128 * g:128 * (g + 1)][:, None])
            rmv = 0.0
        in_maps.append({
            "embb": emb_bf,
            "idx": _wrap_idx(toks.reshape(-1)),
            "wihT": wihT, "whhT": whhT, "sel": sel,
            "brz": brz, "bnn": bnn, "mask": m,
            "rmask": np.full((128, 1), rmv, np.float32),
            "rbt": np.full((128, 2, 128), rmv, np.float32).astype(bf),
            "hWT": hWT,
        })
    return in_maps


def prep_mlp(contribs, hidden_b, scorer_W, scorer_b):
    """contribs: list of 8 arrays [A, 128, H] f32 (per encoder core).
    Pure slicing/replication -- no arithmetic."""
    hidden_b = np.asarray(hidden_b, np.float32)
    scorer_W = np.asarray(scorer_W, np.float32)
    scorer_b = np.asarray(scorer_b, np.float32)
    hbt = np.tile(hidden_b.reshape(1, H), (128, 1)).astype(bf)
    scb = np.tile(scorer_W.reshape(1, H), (128, 1)).astype(bf)
    sbt = np.full((128, 1), float(scorer_b.reshape(-1)[0]), np.float32)

    in_maps = []
    for c in range(NCORES):
        quarter = c // 4          # which half-core (0: cores 0/2/4, 1: cores 1/3/5)
        row0 = 32 * (c % 4)       # state rows within that core's slot-7 block
        srows = (row0 + np.arange(256) // A)                 # per (s,a) row
        cs = {}
        for name, enc_core in (("cobs", 0), ("clook", 2), ("cinv", 4)):
            src = contribs[enc_core + quarter][A - 1]        # slot 7 [128, H]
            rep = src[srows]                                 # [256, H]
            cs[name] = np.ascontiguousarray(rep.reshape(2, 128, H).transpose(1, 0, 2)).astype(bf)
        acore = 6 + quarter
        g0 = 2 * (c % 4)
        cact = np.concatenate([contribs[acore][g0], contribs[acore][g0 + 1]],
                              axis=0)                        # [256, H]
        in_maps.append({
            **cs,
            "cact": np.ascontiguousarray(cact.reshape(2, 128, H).transpose(1, 0, 2)).astype(bf),
            "hbt": hbt, "scorerb": scb, "sbt": sbt,
        })
    return in_maps


_NC_CACHE = {}


def kernel(**inputs):
    if "enc" not in _NC_CACHE:
        _NC_CACHE["enc"] = build_enc(1)
        _NC_CACHE["mlp"] = build_mlp(1)
    nc_e, nc_m = _NC_CACHE["enc"], _NC_CACHE["mlp"]

    enc_maps = prep_enc(**inputs)
    res_e = run_bass_kernel_spmd(nc_e, enc_maps, list(range(NCORES)))
    contribs = [np.asarray(res_e.results[c]["contrib"], np.float32)
                for c in range(NCORES)]

    mlp_maps = prep_mlp(contribs, inputs["hidden_b"], inputs["scorer_W"],
                        inputs["scorer_b"])
    res_m = run_bass_kernel_spmd(nc_m, mlp_maps, list(range(NCORES)))
    q = np.concatenate([np.asarray(res_m.results[c]["q"], np.float32).reshape(-1)
                        for c in range(NCORES)])
    return q.reshape(B, A)



# revision 31
# speedup vs baseline: 327.5644x; 327.5644x over previous
"""DRRN scoring network, v5: transposed-layout GRU on 8 NeuronCores.

Phase 1 (encoders): cores 0-1 obs, 2-3 look, 4-5 inv, 6-7 act; each core
runs one GRU over 128 lanes x 256 steps. The recurrence keeps h in
TRANSPOSED layout h^T [H(part), lanes] the whole time, so the per-step
DMA transposes of v4 (the 2-byte-granular xbar ops that dominated its
590 ms/rep) disappear entirely. Gate pre-activations are computed as
[gate(part), lane] blocks: per step, 21 small matmuls (biases via K<=4
indicator tricks, the z-length-mask via a K=1 rank-1 matmul adding -30
to the z pre-activation of finished lanes, which freezes h exactly).
Embedding rows are gathered contiguously (256B/token, no gather
transpose) and flipped to x^T with PE transposes, pipelined one slab
(32 steps) ahead. nreps runs through a For_i hardware loop so the NEFF
size is independent of nreps.

Phase 2 (MLP): unchanged from v4 -- data-parallel over (state, action)
rows; host only slices/replicates phase-1 outputs between dispatches.
"""
import numpy as np
import ml_dtypes
import concourse.bacc as bacc
import concourse.mybir as mybir
from concourse.tile import TileContext
from concourse.bass_utils import run_bass_kernel_spmd

dt = mybir.dt
F32, BF16, I16 = dt.float32, dt.bfloat16, dt.int16
bf = ml_dtypes.bfloat16

V, E, H = 32000, 128, 256
B, S = 256, 256
A, SA = 8, 32
NCORES = 8
NSTEP = S
NIDX = NSTEP * 128
NSLAB = 8
SLAB = NIDX // NSLAB          # 4096 tokens per slab
H3, H2 = 3 * H, 2 * H
BIG = 30.0

Sig = mybir.ActivationFunctionType.Sigmoid
Tanh = mybir.ActivationFunctionType.Tanh
Relu = mybir.ActivationFunctionType.Relu
Ident = mybir.ActivationFunctionType.Identity
MUL = mybir.AluOpType.mult
ADD = mybir.AluOpType.add
SUB = mybir.AluOpType.subtract


def build_enc(nreps=1, dbg=None):
    dbg = dbg or {}
    n_slab = dbg.get("nslab", NSLAB)
    use_gather = not dbg.get("nogather", False)
    use_mask = not dbg.get("nomask", False)
    use_chain = not dbg.get("nochain", False)
    nc = bacc.Bacc("TRN2", target_bir_lowering=False, debug=False)

    d_emb = nc.declare_dram_parameter("embb", [V, E], BF16, isOutput=False)
    d_idx = nc.declare_dram_parameter("idx", [128, NIDX // 16], I16, isOutput=False)
    d_wih = nc.declare_dram_parameter("wihT", [E, H3], BF16, isOutput=False)
    d_whh = nc.declare_dram_parameter("whhT", [128, 2, H3], BF16, isOutput=False)
    d_b4 = nc.declare_dram_parameter("brz4", [4, 128], BF16, isOutput=False)
    d_bng = nc.declare_dram_parameter("bng4", [4, 128], BF16, isOutput=False)
    d_ind = nc.declare_dram_parameter("ind4", [4, 512], BF16, isOutput=False)
    d_bsel = nc.declare_dram_parameter("bigsel", [32, 32, 128], BF16, isOutput=False)
    d_mn = nc.declare_dram_parameter("mn", [32, NSLAB, 256], BF16, isOutput=False)
    d_rst = nc.declare_dram_parameter("rst", [128, 2, 128], BF16, isOutput=False)
    d_hw = nc.declare_dram_parameter("hWT", [128, 2, H], BF16, isOutput=False)
    d_c = nc.declare_dram_parameter("contrib", [A, 128, H], F32, isOutput=True)

    with TileContext(nc) as tc:
        with tc.tile_pool(name="w", bufs=1) as wp, \
             tc.tile_pool(name="x", bufs=1) as xp, \
             tc.tile_pool(name="r", bufs=2) as rp, \
             tc.tile_pool(name="h", bufs=3) as hp, \
             tc.tile_pool(name="ps", bufs=2, space="PSUM") as pp, \
             tc.tile_pool(name="pc", bufs=2, space="PSUM") as qp:

            t_idx = wp.tile([128, NIDX // 16], I16, name="t_idx")
            nc.sync.dma_start(out=t_idx[:], in_=d_idx[:])
            t_wih = wp.tile([E, H3], BF16, name="t_wih")
            nc.sync.dma_start(out=t_wih[:], in_=d_wih[:])
            t_whh = wp.tile([128, 2, H3], BF16, name="t_whh")
            nc.sync.dma_start(out=t_whh[:], in_=d_whh[:])
            t_b4 = wp.tile([4, 128], BF16, name="t_b4")
            nc.sync.dma_start(out=t_b4[:], in_=d_b4[:])
            t_bng = wp.tile([4, 128], BF16, name="t_bng")
            nc.sync.dma_start(out=t_bng[:], in_=d_bng[:])
            t_ind = wp.tile([4, 512], BF16, name="t_ind")
            nc.sync.dma_start(out=t_ind[:], in_=d_ind[:])
            t_bsel = wp.tile([32, 32, 128], BF16, name="t_bsel")
            nc.sync.dma_start(out=t_bsel[:], in_=d_bsel[:])
            t_mn = wp.tile([32, NSLAB, 256], BF16, name="t_mn")
            nc.sync.dma_start(out=t_mn[:], in_=d_mn[:])
            t_rst = wp.tile([128, 2, 128], BF16, name="t_rst")
            nc.sync.dma_start(out=t_rst[:], in_=d_rst[:])
            t_hw = wp.tile([128, 2, H], BF16, name="t_hw")
            nc.sync.dma_start(out=t_hw[:], in_=d_hw[:])

            def gather(slab, xT):
                # x^T gather: out [E(part), 1, SLAB tokens]
                nc.gpsimd.dma_gather(
                    out_ap=xT[:], in_ap=d_emb[:],
                    idxs_ap=t_idx[:, (SLAB // 16) * slab:(SLAB // 16) * (slab + 1)],
                    num_idxs=SLAB, num_idxs_reg=SLAB, elem_size=E,
                    transpose=True, single_packet=False,
                )

            def body(_iv=None):
                xT = {}
                if use_gather:
                    for s in range(n_slab):
                        xT[s] = xp.tile([128, 1, SLAB], BF16, tag=f"xT{s}",
                                        name=f"xT{s}")
                        gather(s, xT[s])

                h = hp.tile([128, 2, 128], BF16, tag="hT", name="hT")
                nc.vector.memset(h[:], 0.0)

                for s in range(n_slab):
                    for t_loc in range(SA):
                        t = SA * s + t_loc
                        if use_gather:
                            xcol = xT[s][:, 0, 128 * t_loc:128 * (t_loc + 1)]
                        else:
                            xcol = t_whh[:, 1, 0:128]

                        prz = pp.tile([128, 512], F32, tag="prz", name="prz")
                        png = pp.tile([128, 512], F32, tag="png", name="png")
                        # ---- matmuls independent of h (can run early) ----
                        nc.tensor.matmul(prz[:], t_b4[:], t_ind[:],
                                         start=True, stop=False,
                                         skip_group_check=True)
                        if use_mask:
                            # selector row t%32 picks step t's mask row; K=32
                            nc.tensor.matmul(prz[:, 256:512],
                                             t_bsel[:, t % 32, :],
                                             t_mn[:, t // 32, :],
                                             start=False, stop=False,
                                             skip_group_check=True)
                        for m in range(4):
                            nc.tensor.matmul(prz[:, 128 * m:128 * (m + 1)],
                                             t_wih[:, 128 * m:128 * (m + 1)], xcol,
                                             start=False, stop=False,
                                             skip_group_check=True)
                        nc.tensor.matmul(png[:], t_bng[:], t_ind[:],
                                         start=True, stop=False,
                                         skip_group_check=True)
                        for j in range(2):
                            nc.tensor.matmul(png[:, 256 + 128 * j:256 + 128 * (j + 1)],
                                             t_wih[:, 512 + 128 * j:512 + 128 * (j + 1)],
                                             xcol, start=False, stop=True,
                                             skip_group_check=True)
                        # ---- h-dependent matmuls (critical path) ----
                        for k in range(2):
                            hk = h[:, k, :]
                            last = k == 1
                            for m in range(4):
                                nc.tensor.matmul(prz[:, 128 * m:128 * (m + 1)],
                                                 t_whh[:, k, 128 * m:128 * (m + 1)],
                                                 hk, start=False, stop=last,
                                                 skip_group_check=True)
                            for j in range(2):
                                nc.tensor.matmul(png[:, 128 * j:128 * (j + 1)],
                                                 t_whh[:, k, 512 + 128 * j:512 + 128 * (j + 1)],
                                                 hk, start=False, stop=last,
                                                 skip_group_check=True)
                        # ---- elementwise chain ----
                        if not use_chain:
                            h2 = hp.tile([128, 2, 128], BF16, tag="hT", name="hT")
                            nc.scalar.activation(h2[:, :, :], prz[:, 0:256], Tanh)
                        else:
                            s_rz = rp.tile([128, 512], BF16, tag="srz", name="s_rz")
                            nc.scalar.activation(s_rz[:], prz[:], Sig)
                            t1 = rp.tile([128, 256], BF16, tag="t1", name="t1")
                            nc.vector.tensor_tensor(t1[:], s_rz[:, 0:256],
                                                    png[:, 0:256], MUL)
                            t2 = rp.tile([128, 256], BF16, tag="t2", name="t2")
                            nc.vector.tensor_tensor(t2[:], t1[:], png[:, 256:512], ADD)
                            s_n = rp.tile([128, 256], BF16, tag="sn", name="s_n")
                            nc.scalar.activation(s_n[:], t2[:], Tanh)
                            d_ = rp.tile([128, 256], BF16, tag="d", name="d_")
                            nc.vector.tensor_tensor(d_[:], s_n[:], h[:, :, :], SUB)
                            u = rp.tile([128, 256], BF16, tag="u", name="u")
                            nc.vector.tensor_tensor(u[:], s_rz[:, 256:512], d_[:], MUL)
                            h2 = hp.tile([128, 2, 128], BF16, tag="hT", name="hT")
                            nc.vector.tensor_tensor(h2[:, :, :], h[:, :, :], u[:], ADD)

                        if t_loc == SA - 1:
                            g = t // SA
                            pc = qp.tile([128, H], F32, tag="pcc", name="pc")
                            nc.tensor.matmul(pc[:], h2[:, 0, :], t_hw[:, 0, :],
                                             start=True, stop=False)
                            nc.tensor.matmul(pc[:], h2[:, 1, :], t_hw[:, 1, :],
                                             start=False, stop=True)
                            c_s = rp.tile([128, H], F32, tag="cs", name="c_s")
                            nc.scalar.activation(c_s[:], pc[:], Ident)
                            nc.sync.dma_start(out=d_c[g], in_=c_s[:])
                            if t != NSTEP - 1:
                                h3 = hp.tile([128, 2, 128], BF16, tag="hT",
                                             name="hT")
                                nc.vector.tensor_tensor(h3[:, :, :], h2[:, :, :],
                                                        t_rst[:, :, :], MUL)
                                h = h3
                            else:
                                h = h2
                        else:
                            h = h2

            if nreps > 1:
                with tc.For_i(0, nreps, 1):
                    body()
            else:
                body()

    nc.compile()
    return nc


def build_mlp(nreps=1):
    nc = bacc.Bacc("TRN2", target_bir_lowering=False, debug=False)

    d_obs = nc.declare_dram_parameter("cobs", [128, 2, H], BF16, isOutput=False)
    d_look = nc.declare_dram_parameter("clook", [128, 2, H], BF16, isOutput=False)
    d_inv = nc.declare_dram_parameter("cinv", [128, 2, H], BF16, isOutput=False)
    d_act = nc.declare_dram_parameter("cact", [128, 2, H], BF16, isOutput=False)
    d_hbt = nc.declare_dram_parameter("hbt", [128, H], BF16, isOutput=False)
    d_scb = nc.declare_dram_parameter("scorerb", [128, H], BF16, isOutput=False)
    d_sbt = nc.declare_dram_parameter("sbt", [128, 1], F32, isOutput=False)
    d_q = nc.declare_dram_parameter("q", [2, 128], F32, isOutput=True)

    with TileContext(nc) as tc:
        with tc.tile_pool(name="w", bufs=1) as wp, \
             tc.tile_pool(name="rot", bufs=2) as rp:
            t_obs = wp.tile([128, 2, H], BF16, name="t_obs")
            nc.sync.dma_start(out=t_obs[:], in_=d_obs[:])
            t_look = wp.tile([128, 2, H], BF16, name="t_look")
            nc.sync.dma_start(out=t_look[:], in_=d_look[:])
            t_inv = wp.tile([128, 2, H], BF16, name="t_inv")
            nc.sync.dma_start(out=t_inv[:], in_=d_inv[:])
            t_act = wp.tile([128, 2, H], BF16, name="t_act")
            nc.sync.dma_start(out=t_act[:], in_=d_act[:])
            t_hbt = wp.tile([128, H], BF16, name="t_hbt")
            nc.sync.dma_start(out=t_hbt[:], in_=d_hbt[:])
            t_scb = wp.tile([128, H], BF16, name="t_scb")
            nc.sync.dma_start(out=t_scb[:], in_=d_scb[:])
            t_sbt = wp.tile([128, 1], F32, name="t_sbt")
            nc.sync.dma_start(out=t_sbt[:], in_=d_sbt[:])

            def body(_iv=None):
                for k in range(2):
                    a1 = rp.tile([128, H], BF16, tag="a1", name="a1")
                    nc.vector.tensor_tensor(a1[:], t_obs[:, k, :], t_look[:, k, :], ADD)
                    a2 = rp.tile([128, H], BF16, tag="a2", name="a2")
                    nc.vector.tensor_tensor(a2[:], t_inv[:, k, :], t_act[:, k, :], ADD)
                    a3 = rp.tile([128, H], BF16, tag="a3", name="a3")
                    nc.vector.tensor_tensor(a3[:], a1[:], a2[:], ADD)
                    zp = rp.tile([128, H], BF16, tag="zp", name="zp")
                    nc.vector.tensor_tensor(zp[:], a3[:], t_hbt[:], ADD)
                    z = rp.tile([128, H], BF16, tag="z", name="z")
                    nc.scalar.activation(z[:], zp[:], Relu)
                    qm = rp.tile([128, H], F32, tag="qm", name="qm")
                    nc.vector.tensor_tensor(qm[:], z[:], t_scb[:], MUL)
                    qv = rp.tile([128, 1], F32, tag="qv", name="qv")
                    nc.vector.reduce_sum(qv[:], qm[:], axis=mybir.AxisListType.X)
                    qf = rp.tile([128, 1], F32, tag="qf", name="qf")
                    nc.vector.tensor_scalar_add(qf[:], qv[:], t_sbt[:, 0:1])
                    nc.sync.dma_start(out=d_q[k], in_=qf[:, 0])

            if nreps > 1:
                with tc.For_i(0, nreps, 1):
                    body()
            else:
                body()

    nc.compile()
    return nc


def _wrap_idx(tokens_flat):
    out = np.zeros((128, NIDX // 16), np.int16)
    for s in range(NSLAB):
        blk = tokens_flat[SLAB * s:SLAB * (s + 1)].reshape(SLAB // 16, 16).T
        out[:, (SLAB // 16) * s:(SLAB // 16) * (s + 1)] = np.tile(blk, (8, 1))
    return out


def prep_enc(obs_tokens, obs_len, look_tokens, look_len, inv_tokens, inv_len,
             act_tokens, act_len, emb, Wih, Whh, bih, bhh,
             hidden_W, hidden_b, scorer_W, scorer_b):
    npf = np.asarray
    enc_tok = [npf(obs_tokens), npf(look_tokens), npf(inv_tokens)]
    enc_len = [np.maximum(npf(obs_len), 1), np.maximum(npf(look_len), 1),
               np.maximum(npf(inv_len), 1)]
    act_tokens = npf(act_tokens)
    act_len = np.maximum(npf(act_len), 1)
    emb = npf(emb, np.float32)
    Wih = npf(Wih, np.float32)
    Whh = npf(Whh, np.float32)
    bih = npf(bih, np.float32)
    bhh = npf(bhh, np.float32)
    hidden_W = npf(hidden_W, np.float32)

    emb_bf = emb.astype(bf)
    ind4 = np.zeros((4, 512), np.float32)
    for k in range(4):
        ind4[k, 128 * k:128 * (k + 1)] = 1.0

    in_maps = []
    for c in range(NCORES):
        e = c // 2
        hf = c % 2
        wihT = np.ascontiguousarray(Wih[e].T).astype(np.float32)    # [E, 768]
        wihT[:, 256:512] *= -1.0
        whhT = np.ascontiguousarray(
            Whh[e].T.reshape(2, 128, H3).transpose(1, 0, 2)).astype(np.float32)
        whhT[:, :, 256:512] *= -1.0
        b_rz = bih[e, 0:H2] + bhh[e, 0:H2]
        brz4 = np.stack([b_rz[0:128], b_rz[128:256],
                         -b_rz[256:384], -b_rz[384:512]])            # [4,128]
        bng4 = np.stack([bhh[e, 512:640], bhh[e, 640:768],
                         bih[e, 512:640], bih[e, 640:768]])          # [4,128]
        hWT = np.ascontiguousarray(
            hidden_W[:, H * e:H * (e + 1)].T.reshape(2, 128, H)
            .transpose(1, 0, 2)).astype(bf)                          # [128,2,H]

        toks = np.zeros((NSTEP, 128), np.int64)
        m = np.zeros((128, NSTEP), np.float32)
        if e < 3:
            seqs = enc_tok[e][128 * hf:128 * (hf + 1)]               # [128, S]
            lens = enc_len[e][128 * hf:128 * (hf + 1)]
            toks[:, :] = seqs.T
            m[:, :] = (np.arange(NSTEP)[None, :] < lens[:, None])
            rmv = 1.0
        else:
            at = act_tokens[1024 * hf:1024 * (hf + 1)]               # [1024, SA]
            al = act_len[1024 * hf:1024 * (hf + 1)]
            for g in range(A):
                toks[SA * g:SA * (g + 1), :] = at[128 * g:128 * (g + 1)].T
                m[:, SA * g:SA * (g + 1)] = (
                    np.arange(SA)[None, :] < al[128 * g:128 * (g + 1)][:, None])
            rmv = 0.0
        # step t's 1-m row lives at partition t%32, column block t//32,
        # duplicated over the two h chunks
        mnot = (1.0 - m).T                                           # [NSTEP, 128]
        mn = np.zeros((32, NSLAB, 256), np.float32)
        for t in range(NSTEP):
            mn[t % 32, t // 32, 0:128] = mnot[t]
            mn[t % 32, t // 32, 128:256] = mnot[t]
        bigsel = np.zeros((32, 32, 128), np.float32)
        for r in range(32):
            bigsel[r, r, :] = -BIG
        in_maps.append({
            "embb": emb_bf,
            "idx": _wrap_idx(toks.reshape(-1)),
            "wihT": wihT.astype(bf), "whhT": whhT.astype(bf),
            "brz4": brz4.astype(bf), "bng4": bng4.astype(bf),
            "ind4": ind4.astype(bf),
            "bigsel": bigsel.astype(bf),
            "mn": mn.astype(bf),
            "rst": np.full((128, 2, 128), rmv, np.float32).astype(bf),
            "hWT": hWT,
        })
    return in_maps


def prep_mlp(contribs, hidden_b, scorer_W, scorer_b):
    """contribs: list of 8 arrays [A, 128, H] f32 (per encoder core).
    Pure slicing/replication -- no arithmetic."""
    hidden_b = np.asarray(hidden_b, np.float32)
    scorer_W = np.asarray(scorer_W, np.float32)
    scorer_b = np.asarray(scorer_b, np.float32)
    hbt = np.tile(hidden_b.reshape(1, H), (128, 1)).astype(bf)
    scb = np.tile(scorer_W.reshape(1, H), (128, 1)).astype(bf)
    sbt = np.full((128, 1), float(scorer_b.reshape(-1)[0]), np.float32)

    in_maps = []
    for c in range(NCORES):
        quarter = c // 4          # which half-core (0: cores 0/2/4, 1: cores 1/3/5)
        row0 = 32 * (c % 4)       # state rows within that core's slot-7 block
        srows = (row0 + np.arange(256) // A)                 # per (s,a) row
        cs = {}
        for name, enc_core in (("cobs", 0), ("clook", 2), ("cinv", 4)):
            src = contribs[enc_core + quarter][A - 1]        # slot 7 [128, H]
            rep = src[srows]                                 # [256, H]
            cs[name] = np.ascontiguousarray(rep.reshape(2, 128, H).transpose(1, 0, 2)).astype(bf)
        acore = 6 + quarter
        g0 = 2 * (c % 4)
        cact = np.concatenate([contribs[acore][g0], contribs[acore][g0 + 1]],
                              axis=0)                        # [256, H]
        in_maps.append({
            **cs,
            "cact": np.ascontiguousarray(cact.reshape(2, 128, H).transpose(1, 0, 2)).astype(bf),
            "hbt": hbt, "scorerb": scb, "sbt": sbt,
        })
    return in_maps


_NC_CACHE = {}


def kernel(**inputs):
    if "enc" not in _NC_CACHE:
        _NC_CACHE["enc"] = build_enc(1)
        _NC_CACHE["mlp"] = build_mlp(1)
    nc_e, nc_m = _NC_CACHE["enc"], _NC_CACHE["mlp"]

    enc_maps = prep_enc(**inputs)
    res_e = run_bass_kernel_spmd(nc_e, enc_maps, list(range(NCORES)))
    contribs = [np.asarray(res_e.results[c]["contrib"], np.float32)
                for c in range(NCORES)]

    mlp_maps = prep_mlp(contribs, inputs["hidden_b"], inputs["scorer_W"],
                        inputs["scorer_b"])
    res_m = run_bass_kernel_spmd(nc_m, mlp_maps, list(range(NCORES)))
    q = np.concatenate([np.asarray(res_m.results[c]["q"], np.float32).reshape(-1)
                        for c in range(NCORES)])
    return q.reshape(B, A)


# revision 36
# speedup vs baseline: 362.6085x; 1.1070x over previous
"""DRRN scoring network, v5: transposed-layout GRU on 8 NeuronCores.

Phase 1 (encoders): cores 0-1 obs, 2-3 look, 4-5 inv, 6-7 act; each core
runs one GRU over 128 lanes x 256 steps. The recurrence keeps h in
TRANSPOSED layout h^T [H(part), lanes] the whole time, so the per-step
DMA transposes of v4 (the 2-byte-granular xbar ops that dominated its
590 ms/rep) disappear entirely. Gate pre-activations are computed as
[gate(part), lane] blocks: per step, 21 small matmuls (biases via K<=4
indicator tricks, the z-length-mask via a K=1 rank-1 matmul adding -30
to the z pre-activation of finished lanes, which freezes h exactly).
Embedding rows are gathered contiguously (256B/token, no gather
transpose) and flipped to x^T with PE transposes, pipelined one slab
(32 steps) ahead. nreps runs through a For_i hardware loop so the NEFF
size is independent of nreps.

Phase 2 (MLP): unchanged from v4 -- data-parallel over (state, action)
rows; host only slices/replicates phase-1 outputs between dispatches.
"""
import numpy as np
import ml_dtypes
import concourse.bacc as bacc
import concourse.mybir as mybir
from concourse.tile import TileContext
from concourse.bass_utils import run_bass_kernel_spmd

dt = mybir.dt
F32, BF16, I16 = dt.float32, dt.bfloat16, dt.int16
bf = ml_dtypes.bfloat16

V, E, H = 32000, 128, 256
B, S = 256, 256
A, SA = 8, 32
NCORES = 8
NSTEP = S
NIDX = NSTEP * 128
NSLAB = 8
SLAB = NIDX // NSLAB          # 4096 tokens per slab
H3, H2 = 3 * H, 2 * H
BIG = 30.0

Sig = mybir.ActivationFunctionType.Sigmoid
Tanh = mybir.ActivationFunctionType.Tanh
Relu = mybir.ActivationFunctionType.Relu
Ident = mybir.ActivationFunctionType.Identity
MUL = mybir.AluOpType.mult
ADD = mybir.AluOpType.add
SUB = mybir.AluOpType.subtract


def build_enc(nreps=1, dbg=None):
    dbg = dbg or {}
    n_slab = dbg.get("nslab", NSLAB)
    use_gather = not dbg.get("nogather", False)
    use_mask = not dbg.get("nomask", False)
    use_chain = not dbg.get("nochain", False)
    nc = bacc.Bacc("TRN2", target_bir_lowering=False, debug=False)

    d_emb = nc.declare_dram_parameter("embb", [V, E], BF16, isOutput=False)
    d_idx = nc.declare_dram_parameter("idx", [128, NIDX // 16], I16, isOutput=False)
    d_wih = nc.declare_dram_parameter("wihT", [E, H3], BF16, isOutput=False)
    d_whh = nc.declare_dram_parameter("whhT", [128, 2, H3], BF16, isOutput=False)
    d_brz = nc.declare_dram_parameter("brz", [2, 2, 128], BF16, isOutput=False)
    d_bng = nc.declare_dram_parameter("bng4", [4, 128], BF16, isOutput=False)
    d_ind = nc.declare_dram_parameter("ind4", [4, 512], BF16, isOutput=False)
    d_id = nc.declare_dram_parameter("ident", [128, 128], BF16, isOutput=False)
    d_bsel = nc.declare_dram_parameter("bigsel", [32, 32, 128], BF16, isOutput=False)
    d_mn = nc.declare_dram_parameter("mn", [32, NSLAB, 256], BF16, isOutput=False)
    d_rst = nc.declare_dram_parameter("rst", [128, 2, 128], BF16, isOutput=False)
    d_hw = nc.declare_dram_parameter("hWT", [128, 2, H], BF16, isOutput=False)
    d_c = nc.declare_dram_parameter("contrib", [A, 128, H], F32, isOutput=True)

    with TileContext(nc) as tc:
        with tc.tile_pool(name="w", bufs=1) as wp, \
             tc.tile_pool(name="x", bufs=1) as xp, \
             tc.tile_pool(name="r", bufs=2) as rp, \
             tc.tile_pool(name="h", bufs=3) as hp, \
             tc.tile_pool(name="ps", bufs=2, space="PSUM") as pp, \
             tc.tile_pool(name="pc", bufs=2, space="PSUM") as qp:

            t_idx = wp.tile([128, NIDX // 16], I16, name="t_idx")
            nc.sync.dma_start(out=t_idx[:], in_=d_idx[:])
            t_wih = wp.tile([E, H3], BF16, name="t_wih")
            nc.sync.dma_start(out=t_wih[:], in_=d_wih[:])
            t_whh = wp.tile([128, 2, H3], BF16, name="t_whh")
            nc.sync.dma_start(out=t_whh[:], in_=d_whh[:])
            t_brz = wp.tile([2, 2, 128], BF16, name="t_brz")
            nc.sync.dma_start(out=t_brz[:], in_=d_brz[:])
            t_id = wp.tile([128, 128], BF16, name="t_id")
            nc.sync.dma_start(out=t_id[:], in_=d_id[:])
            t_bng = wp.tile([4, 128], BF16, name="t_bng")
            nc.sync.dma_start(out=t_bng[:], in_=d_bng[:])
            t_ind = wp.tile([4, 512], BF16, name="t_ind")
            nc.sync.dma_start(out=t_ind[:], in_=d_ind[:])
            t_bsel = wp.tile([32, 32, 128], BF16, name="t_bsel")
            nc.sync.dma_start(out=t_bsel[:], in_=d_bsel[:])
            t_mn = wp.tile([32, NSLAB, 256], BF16, name="t_mn")
            nc.sync.dma_start(out=t_mn[:], in_=d_mn[:])
            t_rst = wp.tile([128, 2, 128], BF16, name="t_rst")
            nc.sync.dma_start(out=t_rst[:], in_=d_rst[:])
            t_hw = wp.tile([128, 2, H], BF16, name="t_hw")
            nc.sync.dma_start(out=t_hw[:], in_=d_hw[:])

            def gather(slab, xT):
                # x^T gather: out [E(part), 1, SLAB tokens]
                nc.gpsimd.dma_gather(
                    out_ap=xT[:], in_ap=d_emb[:],
                    idxs_ap=t_idx[:, (SLAB // 16) * slab:(SLAB // 16) * (slab + 1)],
                    num_idxs=SLAB, num_idxs_reg=SLAB, elem_size=E,
                    transpose=True, single_packet=False,
                )

            def body(_iv=None):
                xT = {}
                if use_gather:
                    for s in range(n_slab):
                        xT[s] = xp.tile([128, 1, SLAB], BF16, tag=f"xT{s}",
                                        name=f"xT{s}")
                        gather(s, xT[s])

                h = hp.tile([128, 2, 128], BF16, tag="hT", name="hT")
                nc.vector.memset(h[:], 0.0)

                for s in range(n_slab):
                    for t_loc in range(SA):
                        t = SA * s + t_loc
                        if use_gather:
                            xcol = xT[s][:, 0, 128 * t_loc:128 * (t_loc + 1)]
                        else:
                            xcol = t_whh[:, 1, 0:128]

                        p_r = pp.tile([128, 256], F32, tag="pr", name="p_r")
                        p_z = pp.tile([128, 256], F32, tag="pz", name="p_z")
                        png = pp.tile([128, 512], F32, tag="png", name="png")
                        # ---- matmuls independent of h (can run early) ----
                        nc.tensor.matmul(p_r[:], t_brz[:, 0, :],
                                         t_ind[0:2, 0:256], start=True,
                                         stop=False, skip_group_check=True)
                        for m in range(2):
                            nc.tensor.matmul(p_r[:, 128 * m:128 * (m + 1)],
                                             t_wih[:, 128 * m:128 * (m + 1)], xcol,
                                             start=False, stop=False,
                                             skip_group_check=True)
                        nc.tensor.matmul(png[:], t_bng[:], t_ind[:],
                                         start=True, stop=False,
                                         skip_group_check=True)
                        for j in range(2):
                            nc.tensor.matmul(png[:, 256 + 128 * j:256 + 128 * (j + 1)],
                                             t_wih[:, 512 + 128 * j:512 + 128 * (j + 1)],
                                             xcol, start=False, stop=False,
                                             skip_group_check=True)
                        nc.tensor.matmul(p_z[:], t_brz[:, 1, :],
                                         t_ind[0:2, 0:256], start=True,
                                         stop=False, skip_group_check=True)
                        if use_mask:
                            # selector row t%32 picks step t's mask row; K=32
                            nc.tensor.matmul(p_z[:], t_bsel[:, t % 32, :],
                                             t_mn[:, t // 32, :],
                                             start=False, stop=False,
                                             skip_group_check=True)
                        for m in range(2):
                            nc.tensor.matmul(p_z[:, 128 * m:128 * (m + 1)],
                                             t_wih[:, 256 + 128 * m:256 + 128 * (m + 1)],
                                             xcol, start=False, stop=False,
                                             skip_group_check=True)
                        # ---- h-dependent matmuls (critical path): r first,
                        # n next, z last (z is only needed at the chain tail)
                        for k in range(2):
                            hk = h[:, k, :]
                            for m in range(2):
                                nc.tensor.matmul(p_r[:, 128 * m:128 * (m + 1)],
                                                 t_whh[:, k, 128 * m:128 * (m + 1)],
                                                 hk, start=False, stop=(k == 1),
                                                 skip_group_check=True)
                        for k in range(2):
                            hk = h[:, k, :]
                            for j in range(2):
                                nc.tensor.matmul(png[:, 128 * j:128 * (j + 1)],
                                                 t_whh[:, k, 512 + 128 * j:512 + 128 * (j + 1)],
                                                 hk, start=False, stop=(k == 1),
                                                 skip_group_check=True)
                        for k in range(2):
                            hk = h[:, k, :]
                            for m in range(2):
                                nc.tensor.matmul(p_z[:, 128 * m:128 * (m + 1)],
                                                 t_whh[:, k, 256 + 128 * m:256 + 128 * (m + 1)],
                                                 hk, start=False, stop=(k == 1),
                                                 skip_group_check=True)
                        # ---- elementwise chain ----
                        if not use_chain:
                            h2 = hp.tile([128, 2, 128], BF16, tag="hT", name="hT")
                            nc.scalar.activation(h2[:, :, :], p_r[:], Tanh)
                        else:
                            s_r = rp.tile([128, 256], BF16, tag="sr", name="s_r")
                            nc.scalar.activation(s_r[:], p_r[:], Sig)
                            t1 = rp.tile([128, 256], BF16, tag="t1", name="t1")
                            nc.vector.tensor_tensor(t1[:], s_r[:],
                                                    png[:, 0:256], MUL)
                            # accumulate t1 into the gi_n psum region on the PE
                            nc.tensor.matmul(png[:, 256:512], t_id[:], t1[:],
                                             start=False, stop=True,
                                             skip_group_check=True)
                            s_zb = rp.tile([128, 256], BF16, tag="szb", name="s_zb")
                            nc.scalar.activation(s_zb[:], p_z[:], Sig)
                            s_n = rp.tile([128, 256], BF16, tag="sn", name="s_n")
                            nc.scalar.activation(s_n[:], png[:, 256:512], Tanh)
                            d_ = rp.tile([128, 256], BF16, tag="d", name="d_")
                            nc.vector.tensor_tensor(d_[:], s_n[:], h[:, :, :], SUB)
                            u = rp.tile([128, 256], BF16, tag="u", name="u")
                            nc.vector.tensor_tensor(u[:], s_zb[:], d_[:], MUL)
                            h2 = hp.tile([128, 2, 128], BF16, tag="hT", name="hT")
                            nc.vector.tensor_tensor(h2[:, :, :], h[:, :, :], u[:], ADD)

                        if t_loc == SA - 1:
                            g = t // SA
                            pc = qp.tile([128, H], F32, tag="pcc", name="pc")
                            nc.tensor.matmul(pc[:], h2[:, 0, :], t_hw[:, 0, :],
                                             start=True, stop=False)
                            nc.tensor.matmul(pc[:], h2[:, 1, :], t_hw[:, 1, :],
                                             start=False, stop=True)
                            c_s = rp.tile([128, H], F32, tag="cs", name="c_s")
                            nc.scalar.activation(c_s[:], pc[:], Ident)
                            nc.sync.dma_start(out=d_c[g], in_=c_s[:])
                            if t != NSTEP - 1:
                                h3 = hp.tile([128, 2, 128], BF16, tag="hT",
                                             name="hT")
                                nc.vector.tensor_tensor(h3[:, :, :], h2[:, :, :],
                                                        t_rst[:, :, :], MUL)
                                h = h3
                            else:
                                h = h2
                        else:
                            h = h2

            if nreps > 1:
                with tc.For_i(0, nreps, 1):
                    body()
            else:
                body()

    nc.compile()
    return nc


def build_mlp(nreps=1):
    nc = bacc.Bacc("TRN2", target_bir_lowering=False, debug=False)

    d_obs = nc.declare_dram_parameter("cobs", [128, 2, H], BF16, isOutput=False)
    d_look = nc.declare_dram_parameter("clook", [128, 2, H], BF16, isOutput=False)
    d_inv = nc.declare_dram_parameter("cinv", [128, 2, H], BF16, isOutput=False)
    d_act = nc.declare_dram_parameter("cact", [128, 2, H], BF16, isOutput=False)
    d_hbt = nc.declare_dram_parameter("hbt", [128, H], BF16, isOutput=False)
    d_scb = nc.declare_dram_parameter("scorerb", [128, H], BF16, isOutput=False)
    d_sbt = nc.declare_dram_parameter("sbt", [128, 1], F32, isOutput=False)
    d_q = nc.declare_dram_parameter("q", [2, 128], F32, isOutput=True)

    with TileContext(nc) as tc:
        with tc.tile_pool(name="w", bufs=1) as wp, \
             tc.tile_pool(name="rot", bufs=2) as rp:
            t_obs = wp.tile([128, 2, H], BF16, name="t_obs")
            nc.sync.dma_start(out=t_obs[:], in_=d_obs[:])
            t_look = wp.tile([128, 2, H], BF16, name="t_look")
            nc.sync.dma_start(out=t_look[:], in_=d_look[:])
            t_inv = wp.tile([128, 2, H], BF16, name="t_inv")
            nc.sync.dma_start(out=t_inv[:], in_=d_inv[:])
            t_act = wp.tile([128, 2, H], BF16, name="t_act")
            nc.sync.dma_start(out=t_act[:], in_=d_act[:])
            t_hbt = wp.tile([128, H], BF16, name="t_hbt")
            nc.sync.dma_start(out=t_hbt[:], in_=d_hbt[:])
            t_scb = wp.tile([128, H], BF16, name="t_scb")
            nc.sync.dma_start(out=t_scb[:], in_=d_scb[:])
            t_sbt = wp.tile([128, 1], F32, name="t_sbt")
            nc.sync.dma_start(out=t_sbt[:], in_=d_sbt[:])

            def body(_iv=None):
                for k in range(2):
                    a1 = rp.tile([128, H], BF16, tag="a1", name="a1")
                    nc.vector.tensor_tensor(a1[:], t_obs[:, k, :], t_look[:, k, :], ADD)
                    a2 = rp.tile([128, H], BF16, tag="a2", name="a2")
                    nc.vector.tensor_tensor(a2[:], t_inv[:, k, :], t_act[:, k, :], ADD)
                    a3 = rp.tile([128, H], BF16, tag="a3", name="a3")
                    nc.vector.tensor_tensor(a3[:], a1[:], a2[:], ADD)
                    zp = rp.tile([128, H], BF16, tag="zp", name="zp")
                    nc.vector.tensor_tensor(zp[:], a3[:], t_hbt[:], ADD)
                    z = rp.tile([128, H], BF16, tag="z", name="z")
                    nc.scalar.activation(z[:], zp[:], Relu)
                    qm = rp.tile([128, H], F32, tag="qm", name="qm")
                    nc.vector.tensor_tensor(qm[:], z[:], t_scb[:], MUL)
                    qv = rp.tile([128, 1], F32, tag="qv", name="qv")
                    nc.vector.reduce_sum(qv[:], qm[:], axis=mybir.AxisListType.X)
                    qf = rp.tile([128, 1], F32, tag="qf", name="qf")
                    nc.vector.tensor_scalar_add(qf[:], qv[:], t_sbt[:, 0:1])
                    nc.sync.dma_start(out=d_q[k], in_=qf[:, 0])

            if nreps > 1:
                with tc.For_i(0, nreps, 1):
                    body()
            else:
                body()

    nc.compile()
    return nc


def _wrap_idx(tokens_flat):
    out = np.zeros((128, NIDX // 16), np.int16)
    for s in range(NSLAB):
        blk = tokens_flat[SLAB * s:SLAB * (s + 1)].reshape(SLAB // 16, 16).T
        out[:, (SLAB // 16) * s:(SLAB // 16) * (s + 1)] = np.tile(blk, (8, 1))
    return out


def prep_enc(obs_tokens, obs_len, look_tokens, look_len, inv_tokens, inv_len,
             act_tokens, act_len, emb, Wih, Whh, bih, bhh,
             hidden_W, hidden_b, scorer_W, scorer_b):
    npf = np.asarray
    enc_tok = [npf(obs_tokens), npf(look_tokens), npf(inv_tokens)]
    enc_len = [np.maximum(npf(obs_len), 1), np.maximum(npf(look_len), 1),
               np.maximum(npf(inv_len), 1)]
    act_tokens = npf(act_tokens)
    act_len = np.maximum(npf(act_len), 1)
    emb = npf(emb, np.float32)
    Wih = npf(Wih, np.float32)
    Whh = npf(Whh, np.float32)
    bih = npf(bih, np.float32)
    bhh = npf(bhh, np.float32)
    hidden_W = npf(hidden_W, np.float32)

    emb_bf = emb.astype(bf)
    ind4 = np.zeros((4, 512), np.float32)
    for k in range(4):
        ind4[k, 128 * k:128 * (k + 1)] = 1.0

    in_maps = []
    for c in range(NCORES):
        e = c // 2
        hf = c % 2
        wihT = np.ascontiguousarray(Wih[e].T).astype(np.float32)    # [E, 768]
        wihT[:, 256:512] *= -1.0
        whhT = np.ascontiguousarray(
            Whh[e].T.reshape(2, 128, H3).transpose(1, 0, 2)).astype(np.float32)
        whhT[:, :, 256:512] *= -1.0
        b_rz = bih[e, 0:H2] + bhh[e, 0:H2]
        brz = np.zeros((2, 2, 128), np.float32)                      # [k, r|z, :]
        brz[0, 0] = b_rz[0:128]
        brz[1, 0] = b_rz[128:256]
        brz[0, 1] = -b_rz[256:384]
        brz[1, 1] = -b_rz[384:512]
        bng4 = np.stack([bhh[e, 512:640], bhh[e, 640:768],
                         bih[e, 512:640], bih[e, 640:768]])          # [4,128]
        hWT = np.ascontiguousarray(
            hidden_W[:, H * e:H * (e + 1)].T.reshape(2, 128, H)
            .transpose(1, 0, 2)).astype(bf)                          # [128,2,H]

        toks = np.zeros((NSTEP, 128), np.int64)
        m = np.zeros((128, NSTEP), np.float32)
        if e < 3:
            seqs = enc_tok[e][128 * hf:128 * (hf + 1)]               # [128, S]
            lens = enc_len[e][128 * hf:128 * (hf + 1)]
            toks[:, :] = seqs.T
            m[:, :] = (np.arange(NSTEP)[None, :] < lens[:, None])
            rmv = 1.0
        else:
            at = act_tokens[1024 * hf:1024 * (hf + 1)]               # [1024, SA]
            al = act_len[1024 * hf:1024 * (hf + 1)]
            for g in range(A):
                toks[SA * g:SA * (g + 1), :] = at[128 * g:128 * (g + 1)].T
                m[:, SA * g:SA * (g + 1)] = (
                    np.arange(SA)[None, :] < al[128 * g:128 * (g + 1)][:, None])
            rmv = 0.0
        # step t's 1-m row lives at partition t%32, column block t//32,
        # duplicated over the two h chunks
        mnot = (1.0 - m).T                                           # [NSTEP, 128]
        mn = np.zeros((32, NSLAB, 256), np.float32)
        for t in range(NSTEP):
            mn[t % 32, t // 32, 0:128] = mnot[t]
            mn[t % 32, t // 32, 128:256] = mnot[t]
        bigsel = np.zeros((32, 32, 128), np.float32)
        for r in range(32):
            bigsel[r, r, :] = -BIG
        in_maps.append({
            "embb": emb_bf,
            "idx": _wrap_idx(toks.reshape(-1)),
            "wihT": wihT.astype(bf), "whhT": whhT.astype(bf),
            "brz": brz.astype(bf), "bng4": bng4.astype(bf),
            "ind4": ind4.astype(bf),
            "ident": np.eye(128, dtype=np.float32).astype(bf),
            "bigsel": bigsel.astype(bf),
            "mn": mn.astype(bf),
            "rst": np.full((128, 2, 128), rmv, np.float32).astype(bf),
            "hWT": hWT,
        })
    return in_maps


def prep_mlp(contribs, hidden_b, scorer_W, scorer_b):
    """contribs: list of 8 arrays [A, 128, H] f32 (per encoder core).
    Pure slicing/replication -- no arithmetic."""
    hidden_b = np.asarray(hidden_b, np.float32)
    scorer_W = np.asarray(scorer_W, np.float32)
    scorer_b = np.asarray(scorer_b, np.float32)
    hbt = np.tile(hidden_b.reshape(1, H), (128, 1)).astype(bf)
    scb = np.tile(scorer_W.reshape(1, H), (128, 1)).astype(bf)
    sbt = np.full((128, 1), float(scorer_b.reshape(-1)[0]), np.float32)

    in_maps = []
    for c in range(NCORES):
        quarter = c // 4          # which half-core (0: cores 0/2/4, 1: cores 1/3/5)
        row0 = 32 * (c % 4)       # state rows within that core's slot-7 block
        srows = (row0 + np.arange(256) // A)                 # per (s,a) row
        cs = {}
        for name, enc_core in (("cobs", 0), ("clook", 2), ("cinv", 4)):
            src = contribs[enc_core + quarter][A - 1]        # slot 7 [128, H]
            rep = src[srows]                                 # [256, H]
            cs[name] = np.ascontiguousarray(rep.reshape(2, 128, H).transpose(1, 0, 2)).astype(bf)
        acore = 6 + quarter
        g0 = 2 * (c % 4)
        cact = np.concatenate([contribs[acore][g0], contribs[acore][g0 + 1]],
                              axis=0)                        # [256, H]
        in_maps.append({
            **cs,
            "cact": np.ascontiguousarray(cact.reshape(2, 128, H).transpose(1, 0, 2)).astype(bf),
            "hbt": hbt, "scorerb": scb, "sbt": sbt,
        })
    return in_maps


_NC_CACHE = {}


def kernel(**inputs):
    if "enc" not in _NC_CACHE:
        _NC_CACHE["enc"] = build_enc(1)
        _NC_CACHE["mlp"] = build_mlp(1)
    nc_e, nc_m = _NC_CACHE["enc"], _NC_CACHE["mlp"]

    enc_maps = prep_enc(**inputs)
    res_e = run_bass_kernel_spmd(nc_e, enc_maps, list(range(NCORES)))
    contribs = [np.asarray(res_e.results[c]["contrib"], np.float32)
                for c in range(NCORES)]

    mlp_maps = prep_mlp(contribs, inputs["hidden_b"], inputs["scorer_W"],
                        inputs["scorer_b"])
    res_m = run_bass_kernel_spmd(nc_m, mlp_maps, list(range(NCORES)))
    q = np.concatenate([np.asarray(res_m.results[c]["q"], np.float32).reshape(-1)
                        for c in range(NCORES)])
    return q.reshape(B, A)


# revision 43
# speedup vs baseline: 436.0987x; 1.2027x over previous
"""DRRN scoring network, v6: transposed-layout GRU on 8 NeuronCores.

Phase 1 (encoders): cores 0-1 obs, 2-3 look, 4-5 inv, 6-7 act; each core
runs one GRU over 128 lanes x 256 steps. The recurrence keeps h in
TRANSPOSED layout h^T [H(part), lanes] the whole time, so the per-step
DMA transposes of v4 (the 2-byte-granular xbar ops that dominated its
590 ms/rep) disappear entirely. Gate pre-activations are computed as
[gate(part), lane] blocks in three PSUM banks split by when the chain
needs them (r early, n middle, z last): ~23 small matmuls per step.
Biases enter via K<=4 indicator matmuls; the variable-length mask is a
K=32 selector matmul adding -30 to the (negated) z pre-activation of
finished lanes, which freezes h exactly (zb=sigmoid(-pre-30)~=0). All
matmul operands stay at base partition 0 -- alternating base partitions
across small-K matmuls hangs the PE. The r*gh_n product is accumulated
back into the gi_n PSUM region by an identity matmul so tanh reads one
PSUM operand. Embeddings arrive via gpsimd dma_gather transpose=True
(direct x^T layout), one slab (32 steps) ahead. nreps runs through a
For_i hardware loop so NEFF size is independent of nreps.

Phase 2 (MLP): unchanged from v4 -- data-parallel over (state, action)
rows; host only slices/replicates phase-1 outputs between dispatches.
"""
import numpy as np
import ml_dtypes
import concourse.bacc as bacc
import concourse.mybir as mybir
from concourse.tile import TileContext
from concourse.bass_utils import run_bass_kernel_spmd

dt = mybir.dt
F32, BF16, I16 = dt.float32, dt.bfloat16, dt.int16
bf = ml_dtypes.bfloat16

V, E, H = 32000, 128, 256
B, S = 256, 256
A, SA = 8, 32
NCORES = 8
NSTEP = S
NIDX = NSTEP * 128
NSLAB = 8
SLAB = NIDX // NSLAB          # 4096 tokens per slab
H3, H2 = 3 * H, 2 * H
BIG = 30.0

Sig = mybir.ActivationFunctionType.Sigmoid
Tanh = mybir.ActivationFunctionType.Tanh
Relu = mybir.ActivationFunctionType.Relu
Ident = mybir.ActivationFunctionType.Identity
MUL = mybir.AluOpType.mult
ADD = mybir.AluOpType.add
SUB = mybir.AluOpType.subtract


def build_enc(nreps=1, dbg=None):
    dbg = dbg or {}
    n_slab = dbg.get("nslab", NSLAB)
    use_gather = not dbg.get("nogather", False)
    use_mask = not dbg.get("nomask", False)
    use_chain = not dbg.get("nochain", False)
    use_inj = not dbg.get("noinj", False)
    csplit = dbg.get("csplit", False)
    rbufs = dbg.get("rbufs", 2)
    hbufs = dbg.get("hbufs", 3)
    nc = bacc.Bacc("TRN2", target_bir_lowering=False, debug=False)

    d_emb = nc.declare_dram_parameter("embb", [V, E], BF16, isOutput=False)
    d_idx = nc.declare_dram_parameter("idx", [128, NIDX // 16], I16, isOutput=False)
    d_wih = nc.declare_dram_parameter("wihT", [E, H3], BF16, isOutput=False)
    d_whh = nc.declare_dram_parameter("whhT", [128, 2, H3], BF16, isOutput=False)
    d_brz = nc.declare_dram_parameter("brz", [2, 2, 128], BF16, isOutput=False)
    d_bng = nc.declare_dram_parameter("bng4", [4, 128], BF16, isOutput=False)
    d_ind = nc.declare_dram_parameter("ind4", [4, 512], BF16, isOutput=False)
    d_id = nc.declare_dram_parameter("ident", [128, 128], BF16, isOutput=False)
    d_bsel = nc.declare_dram_parameter("bigsel", [32, 32, 128], BF16, isOutput=False)
    d_mn = nc.declare_dram_parameter("mn", [32, NSLAB, 256], BF16, isOutput=False)
    d_rst = nc.declare_dram_parameter("rst", [128, 2, 128], BF16, isOutput=False)
    d_hw = nc.declare_dram_parameter("hWT", [128, 2, H], BF16, isOutput=False)
    d_c = nc.declare_dram_parameter("contrib", [A, 128, H], F32, isOutput=True)

    with TileContext(nc) as tc:
        with tc.tile_pool(name="w", bufs=1) as wp, \
             tc.tile_pool(name="x", bufs=1) as xp, \
             tc.tile_pool(name="r", bufs=rbufs) as rp, \
             tc.tile_pool(name="h", bufs=hbufs) as hp, \
             tc.tile_pool(name="ps", bufs=2, space="PSUM") as pp, \
             tc.tile_pool(name="pc", bufs=2, space="PSUM") as qp:

            t_idx = wp.tile([128, NIDX // 16], I16, name="t_idx")
            nc.sync.dma_start(out=t_idx[:], in_=d_idx[:])
            t_wih = wp.tile([E, H3], BF16, name="t_wih")
            nc.sync.dma_start(out=t_wih[:], in_=d_wih[:])
            t_whh = wp.tile([128, 2, H3], BF16, name="t_whh")
            nc.sync.dma_start(out=t_whh[:], in_=d_whh[:])
            t_brz = wp.tile([2, 2, 128], BF16, name="t_brz")
            nc.sync.dma_start(out=t_brz[:], in_=d_brz[:])
            t_id = wp.tile([128, 128], BF16, name="t_id")
            nc.sync.dma_start(out=t_id[:], in_=d_id[:])
            t_bng = wp.tile([4, 128], BF16, name="t_bng")
            nc.sync.dma_start(out=t_bng[:], in_=d_bng[:])
            t_ind = wp.tile([4, 512], BF16, name="t_ind")
            nc.sync.dma_start(out=t_ind[:], in_=d_ind[:])
            t_bsel = wp.tile([32, 32, 128], BF16, name="t_bsel")
            nc.sync.dma_start(out=t_bsel[:], in_=d_bsel[:])
            t_mn = wp.tile([32, NSLAB, 256], BF16, name="t_mn")
            nc.sync.dma_start(out=t_mn[:], in_=d_mn[:])
            t_rst = wp.tile([128, 2, 128], BF16, name="t_rst")
            nc.sync.dma_start(out=t_rst[:], in_=d_rst[:])
            t_hw = wp.tile([128, 2, H], BF16, name="t_hw")
            nc.sync.dma_start(out=t_hw[:], in_=d_hw[:])

            def gather(slab, xT):
                # x^T gather: out [E(part), 1, SLAB tokens]
                nc.gpsimd.dma_gather(
                    out_ap=xT[:], in_ap=d_emb[:],
                    idxs_ap=t_idx[:, (SLAB // 16) * slab:(SLAB // 16) * (slab + 1)],
                    num_idxs=SLAB, num_idxs_reg=SLAB, elem_size=E,
                    transpose=True, single_packet=False,
                )

            def body(_iv=None):
                xT = {}
                if use_gather:
                    for s in range(n_slab):
                        xT[s] = xp.tile([128, 1, SLAB], BF16, tag=f"xT{s}",
                                        name=f"xT{s}")
                        gather(s, xT[s])

                h = hp.tile([128, 2, 128], BF16, tag="hT", name="hT")
                nc.vector.memset(h[:], 0.0)

                for s in range(n_slab):
                    for t_loc in range(SA):
                        t = SA * s + t_loc
                        if use_gather:
                            xcol = xT[s][:, 0, 128 * t_loc:128 * (t_loc + 1)]
                        else:
                            xcol = t_whh[:, 1, 0:128]

                        p_r = pp.tile([128, 256], F32, tag="pr", name="p_r")
                        p_z = pp.tile([128, 256], F32, tag="pz", name="p_z")
                        png = pp.tile([128, 512], F32, tag="png", name="png")
                        # ---- matmuls independent of h (can run early) ----
                        nc.tensor.matmul(p_r[:], t_brz[:, 0, :],
                                         t_ind[0:2, 0:256], start=True,
                                         stop=False, skip_group_check=True)
                        for m in range(2):
                            nc.tensor.matmul(p_r[:, 128 * m:128 * (m + 1)],
                                             t_wih[:, 128 * m:128 * (m + 1)], xcol,
                                             start=False, stop=False,
                                             skip_group_check=True)
                        nc.tensor.matmul(png[:], t_bng[:], t_ind[:],
                                         start=True, stop=False,
                                         skip_group_check=True)
                        for j in range(2):
                            nc.tensor.matmul(png[:, 256 + 128 * j:256 + 128 * (j + 1)],
                                             t_wih[:, 512 + 128 * j:512 + 128 * (j + 1)],
                                             xcol, start=False, stop=False,
                                             skip_group_check=True)
                        nc.tensor.matmul(p_z[:], t_brz[:, 1, :],
                                         t_ind[0:2, 0:256], start=True,
                                         stop=False, skip_group_check=True)
                        if use_mask:
                            # selector row t%32 picks step t's mask row; K=32
                            nc.tensor.matmul(p_z[:], t_bsel[:, t % 32, :],
                                             t_mn[:, t // 32, :],
                                             start=False, stop=False,
                                             skip_group_check=True)
                        for m in range(2):
                            nc.tensor.matmul(p_z[:, 128 * m:128 * (m + 1)],
                                             t_wih[:, 256 + 128 * m:256 + 128 * (m + 1)],
                                             xcol, start=False, stop=False,
                                             skip_group_check=True)
                        # ---- h-dependent matmuls (critical path): r first,
                        # n next, z last (z is only needed at the chain tail)
                        for k in range(2):
                            hk = h[:, k, :]
                            for m in range(2):
                                nc.tensor.matmul(p_r[:, 128 * m:128 * (m + 1)],
                                                 t_whh[:, k, 128 * m:128 * (m + 1)],
                                                 hk, start=False, stop=(k == 1),
                                                 skip_group_check=True)
                        for k in range(2):
                            hk = h[:, k, :]
                            for j in range(2):
                                nc.tensor.matmul(png[:, 128 * j:128 * (j + 1)],
                                                 t_whh[:, k, 512 + 128 * j:512 + 128 * (j + 1)],
                                                 hk, start=False, stop=(k == 1),
                                                 skip_group_check=True)
                        for k in range(2):
                            hk = h[:, k, :]
                            for m in range(2):
                                nc.tensor.matmul(p_z[:, 128 * m:128 * (m + 1)],
                                                 t_whh[:, k, 256 + 128 * m:256 + 128 * (m + 1)],
                                                 hk, start=False, stop=(k == 1),
                                                 skip_group_check=True)
                        # ---- elementwise chain ----
                        if not use_chain:
                            h2 = hp.tile([128, 2, 128], BF16, tag="hT", name="hT")
                            nc.scalar.activation(h2[:, :, :], p_r[:], Tanh)
                        else:
                            s_r = rp.tile([128, 256], BF16, tag="sr", name="s_r")
                            nc.scalar.activation(s_r[:], p_r[:], Sig)
                            t1 = rp.tile([128, 256], BF16, tag="t1", name="t1")
                            nc.vector.tensor_tensor(t1[:], s_r[:],
                                                    png[:, 0:256], MUL)
                            if use_inj:
                                # accumulate t1 into the gi_n region on the PE
                                nc.tensor.matmul(png[:, 256:512], t_id[:], t1[:],
                                                 start=False, stop=True,
                                                 skip_group_check=True)
                            else:
                                t2 = rp.tile([128, 256], BF16, tag="t2",
                                             name="t2")
                                nc.vector.tensor_tensor(t2[:], t1[:],
                                                        png[:, 256:512], ADD)
                            s_zb = rp.tile([128, 256], BF16, tag="szb", name="s_zb")
                            nc.scalar.activation(s_zb[:], p_z[:], Sig)
                            s_n = rp.tile([128, 256], BF16, tag="sn", name="s_n")
                            nsrc = png[:, 256:512] if use_inj else t2[:]
                            h2 = hp.tile([128, 2, 128], BF16, tag="hT", name="hT")
                            if csplit:
                                # chunk-pipelined tail: h2 chunk 0 lands early
                                # so next step's k=0 matmuls can begin
                                for j in range(2):
                                    cj = slice(128 * j, 128 * (j + 1))
                                    nc.scalar.activation(s_n[:, cj], nsrc[:, cj],
                                                         Tanh)
                                    d_ = rp.tile([128, 128], BF16, tag=f"d{j}",
                                                 name="d_")
                                    nc.vector.tensor_tensor(d_[:], s_n[:, cj],
                                                            h[:, j, :], SUB)
                                    u = rp.tile([128, 128], BF16, tag=f"u{j}",
                                                name="u")
                                    nc.vector.tensor_tensor(u[:], s_zb[:, cj],
                                                            d_[:], MUL)
                                    nc.vector.tensor_tensor(h2[:, j, :],
                                                            h[:, j, :], u[:], ADD)
                            else:
                                nc.scalar.activation(s_n[:], nsrc, Tanh)
                                d_ = rp.tile([128, 256], BF16, tag="d", name="d_")
                                nc.vector.tensor_tensor(d_[:], s_n[:], h[:, :, :],
                                                        SUB)
                                u = rp.tile([128, 256], BF16, tag="u", name="u")
                                nc.vector.tensor_tensor(u[:], s_zb[:], d_[:], MUL)
                                nc.vector.tensor_tensor(h2[:, :, :], h[:, :, :],
                                                        u[:], ADD)

                        if t_loc == SA - 1:
                            g = t // SA
                            pc = qp.tile([128, H], F32, tag="pcc", name="pc")
                            nc.tensor.matmul(pc[:], h2[:, 0, :], t_hw[:, 0, :],
                                             start=True, stop=False)
                            nc.tensor.matmul(pc[:], h2[:, 1, :], t_hw[:, 1, :],
                                             start=False, stop=True)
                            c_s = rp.tile([128, H], F32, tag="cs", name="c_s")
                            nc.scalar.activation(c_s[:], pc[:], Ident)
                            nc.sync.dma_start(out=d_c[g], in_=c_s[:])
                            if t != NSTEP - 1:
                                h3 = hp.tile([128, 2, 128], BF16, tag="hT",
                                             name="hT")
                                nc.vector.tensor_tensor(h3[:, :, :], h2[:, :, :],
                                                        t_rst[:, :, :], MUL)
                                h = h3
                            else:
                                h = h2
                        else:
                            h = h2

            if nreps > 1:
                with tc.For_i(0, nreps, 1):
                    body()
            else:
                body()

    nc.compile()
    return nc


def build_mlp(nreps=1):
    nc = bacc.Bacc("TRN2", target_bir_lowering=False, debug=False)

    d_obs = nc.declare_dram_parameter("cobs", [128, 2, H], BF16, isOutput=False)
    d_look = nc.declare_dram_parameter("clook", [128, 2, H], BF16, isOutput=False)
    d_inv = nc.declare_dram_parameter("cinv", [128, 2, H], BF16, isOutput=False)
    d_act = nc.declare_dram_parameter("cact", [128, 2, H], BF16, isOutput=False)
    d_hbt = nc.declare_dram_parameter("hbt", [128, H], BF16, isOutput=False)
    d_scb = nc.declare_dram_parameter("scorerb", [128, H], BF16, isOutput=False)
    d_sbt = nc.declare_dram_parameter("sbt", [128, 1], F32, isOutput=False)
    d_q = nc.declare_dram_parameter("q", [2, 128], F32, isOutput=True)

    with TileContext(nc) as tc:
        with tc.tile_pool(name="w", bufs=1) as wp, \
             tc.tile_pool(name="rot", bufs=2) as rp:
            t_obs = wp.tile([128, 2, H], BF16, name="t_obs")
            nc.sync.dma_start(out=t_obs[:], in_=d_obs[:])
            t_look = wp.tile([128, 2, H], BF16, name="t_look")
            nc.sync.dma_start(out=t_look[:], in_=d_look[:])
            t_inv = wp.tile([128, 2, H], BF16, name="t_inv")
            nc.sync.dma_start(out=t_inv[:], in_=d_inv[:])
            t_act = wp.tile([128, 2, H], BF16, name="t_act")
            nc.sync.dma_start(out=t_act[:], in_=d_act[:])
            t_hbt = wp.tile([128, H], BF16, name="t_hbt")
            nc.sync.dma_start(out=t_hbt[:], in_=d_hbt[:])
            t_scb = wp.tile([128, H], BF16, name="t_scb")
            nc.sync.dma_start(out=t_scb[:], in_=d_scb[:])
            t_sbt = wp.tile([128, 1], F32, name="t_sbt")
            nc.sync.dma_start(out=t_sbt[:], in_=d_sbt[:])

            def body(_iv=None):
                for k in range(2):
                    a1 = rp.tile([128, H], BF16, tag="a1", name="a1")
                    nc.vector.tensor_tensor(a1[:], t_obs[:, k, :], t_look[:, k, :], ADD)
                    a2 = rp.tile([128, H], BF16, tag="a2", name="a2")
                    nc.vector.tensor_tensor(a2[:], t_inv[:, k, :], t_act[:, k, :], ADD)
                    a3 = rp.tile([128, H], BF16, tag="a3", name="a3")
                    nc.vector.tensor_tensor(a3[:], a1[:], a2[:], ADD)
                    zp = rp.tile([128, H], BF16, tag="zp", name="zp")
                    nc.vector.tensor_tensor(zp[:], a3[:], t_hbt[:], ADD)
                    z = rp.tile([128, H], BF16, tag="z", name="z")
                    nc.scalar.activation(z[:], zp[:], Relu)
                    qm = rp.tile([128, H], F32, tag="qm", name="qm")
                    nc.vector.tensor_tensor(qm[:], z[:], t_scb[:], MUL)
                    qv = rp.tile([128, 1], F32, tag="qv", name="qv")
                    nc.vector.reduce_sum(qv[:], qm[:], axis=mybir.AxisListType.X)
                    qf = rp.tile([128, 1], F32, tag="qf", name="qf")
                    nc.vector.tensor_scalar_add(qf[:], qv[:], t_sbt[:, 0:1])
                    nc.sync.dma_start(out=d_q[k], in_=qf[:, 0])

            if nreps > 1:
                with tc.For_i(0, nreps, 1):
                    body()
            else:
                body()

    nc.compile()
    return nc


def _wrap_idx(tokens_flat):
    out = np.zeros((128, NIDX // 16), np.int16)
    for s in range(NSLAB):
        blk = tokens_flat[SLAB * s:SLAB * (s + 1)].reshape(SLAB // 16, 16).T
        out[:, (SLAB // 16) * s:(SLAB // 16) * (s + 1)] = np.tile(blk, (8, 1))
    return out


def prep_enc(obs_tokens, obs_len, look_tokens, look_len, inv_tokens, inv_len,
             act_tokens, act_len, emb, Wih, Whh, bih, bhh,
             hidden_W, hidden_b, scorer_W, scorer_b):
    npf = np.asarray
    enc_tok = [npf(obs_tokens), npf(look_tokens), npf(inv_tokens)]
    enc_len = [np.maximum(npf(obs_len), 1), np.maximum(npf(look_len), 1),
               np.maximum(npf(inv_len), 1)]
    act_tokens = npf(act_tokens)
    act_len = np.maximum(npf(act_len), 1)
    emb = npf(emb, np.float32)
    Wih = npf(Wih, np.float32)
    Whh = npf(Whh, np.float32)
    bih = npf(bih, np.float32)
    bhh = npf(bhh, np.float32)
    hidden_W = npf(hidden_W, np.float32)

    emb_bf = emb.astype(bf)
    ind4 = np.zeros((4, 512), np.float32)
    for k in range(4):
        ind4[k, 128 * k:128 * (k + 1)] = 1.0

    in_maps = []
    for c in range(NCORES):
        e = c // 2
        hf = c % 2
        wihT = np.ascontiguousarray(Wih[e].T).astype(np.float32)    # [E, 768]
        wihT[:, 256:512] *= -1.0
        whhT = np.ascontiguousarray(
            Whh[e].T.reshape(2, 128, H3).transpose(1, 0, 2)).astype(np.float32)
        whhT[:, :, 256:512] *= -1.0
        b_rz = bih[e, 0:H2] + bhh[e, 0:H2]
        brz = np.zeros((2, 2, 128), np.float32)                      # [k, r|z, :]
        brz[0, 0] = b_rz[0:128]
        brz[1, 0] = b_rz[128:256]
        brz[0, 1] = -b_rz[256:384]
        brz[1, 1] = -b_rz[384:512]
        bng4 = np.stack([bhh[e, 512:640], bhh[e, 640:768],
                         bih[e, 512:640], bih[e, 640:768]])          # [4,128]
        hWT = np.ascontiguousarray(
            hidden_W[:, H * e:H * (e + 1)].T.reshape(2, 128, H)
            .transpose(1, 0, 2)).astype(bf)                          # [128,2,H]

        toks = np.zeros((NSTEP, 128), np.int64)
        m = np.zeros((128, NSTEP), np.float32)
        if e < 3:
            seqs = enc_tok[e][128 * hf:128 * (hf + 1)]               # [128, S]
            lens = enc_len[e][128 * hf:128 * (hf + 1)]
            toks[:, :] = seqs.T
            m[:, :] = (np.arange(NSTEP)[None, :] < lens[:, None])
            rmv = 1.0
        else:
            at = act_tokens[1024 * hf:1024 * (hf + 1)]               # [1024, SA]
            al = act_len[1024 * hf:1024 * (hf + 1)]
            for g in range(A):
                toks[SA * g:SA * (g + 1), :] = at[128 * g:128 * (g + 1)].T
                m[:, SA * g:SA * (g + 1)] = (
                    np.arange(SA)[None, :] < al[128 * g:128 * (g + 1)][:, None])
            rmv = 0.0
        # step t's 1-m row lives at partition t%32, column block t//32,
        # duplicated over the two h chunks
        mnot = (1.0 - m).T                                           # [NSTEP, 128]
        mn = np.zeros((32, NSLAB, 256), np.float32)
        for t in range(NSTEP):
            mn[t % 32, t // 32, 0:128] = mnot[t]
            mn[t % 32, t // 32, 128:256] = mnot[t]
        bigsel = np.zeros((32, 32, 128), np.float32)
        for r in range(32):
            bigsel[r, r, :] = -BIG
        in_maps.append({
            "embb": emb_bf,
            "idx": _wrap_idx(toks.reshape(-1)),
            "wihT": wihT.astype(bf), "whhT": whhT.astype(bf),
            "brz": brz.astype(bf), "bng4": bng4.astype(bf),
            "ind4": ind4.astype(bf),
            "ident": np.eye(128, dtype=np.float32).astype(bf),
            "bigsel": bigsel.astype(bf),
            "mn": mn.astype(bf),
            "rst": np.full((128, 2, 128), rmv, np.float32).astype(bf),
            "hWT": hWT,
        })
    return in_maps


def prep_mlp(contribs, hidden_b, scorer_W, scorer_b):
    """contribs: list of 8 arrays [A, 128, H] f32 (per encoder core).
    Pure slicing/replication -- no arithmetic."""
    hidden_b = np.asarray(hidden_b, np.float32)
    scorer_W = np.asarray(scorer_W, np.float32)
    scorer_b = np.asarray(scorer_b, np.float32)
    hbt = np.tile(hidden_b.reshape(1, H), (128, 1)).astype(bf)
    scb = np.tile(scorer_W.reshape(1, H), (128, 1)).astype(bf)
    sbt = np.full((128, 1), float(scorer_b.reshape(-1)[0]), np.float32)

    in_maps = []
    for c in range(NCORES):
        quarter = c // 4          # which half-core (0: cores 0/2/4, 1: cores 1/3/5)
        row0 = 32 * (c % 4)       # state rows within that core's slot-7 block
        srows = (row0 + np.arange(256) // A)                 # per (s,a) row
        cs = {}
        for name, enc_core in (("cobs", 0), ("clook", 2), ("cinv", 4)):
            src = contribs[enc_core + quarter][A - 1]        # slot 7 [128, H]
            rep = src[srows]                                 # [256, H]
            cs[name] = np.ascontiguousarray(rep.reshape(2, 128, H).transpose(1, 0, 2)).astype(bf)
        acore = 6 + quarter
        g0 = 2 * (c % 4)
        cact = np.concatenate([contribs[acore][g0], contribs[acore][g0 + 1]],
                              axis=0)                        # [256, H]
        in_maps.append({
            **cs,
            "cact": np.ascontiguousarray(cact.reshape(2, 128, H).transpose(1, 0, 2)).astype(bf),
            "hbt": hbt, "scorerb": scb, "sbt": sbt,
        })
    return in_maps


_NC_CACHE = {}


def kernel(**inputs):
    if "enc" not in _NC_CACHE:
        _NC_CACHE["enc"] = build_enc(1)
        _NC_CACHE["mlp"] = build_mlp(1)
    nc_e, nc_m = _NC_CACHE["enc"], _NC_CACHE["mlp"]

    enc_maps = prep_enc(**inputs)
    res_e = run_bass_kernel_spmd(nc_e, enc_maps, list(range(NCORES)))
    contribs = [np.asarray(res_e.results[c]["contrib"], np.float32)
                for c in range(NCORES)]

    mlp_maps = prep_mlp(contribs, inputs["hidden_b"], inputs["scorer_W"],
                        inputs["scorer_b"])
    res_m = run_bass_kernel_spmd(nc_m, mlp_maps, list(range(NCORES)))
    q = np.concatenate([np.asarray(res_m.results[c]["q"], np.float32).reshape(-1)
                        for c in range(NCORES)])
    return q.reshape(B, A)
